# revision 2
# baseline (speedup 1.0000x reference)
"""Multi-head attention TRN2 Bass kernel, head-sharded across 8 NeuronCores.

Problem: S=2048, E=1024, H=16 heads, dk=dv=64, fp32.
    Q = x @ Wq.T ; K = x @ Wk.T ; V = x @ Wv.T   (per-head slices)
    A_h = softmax(Q_h K_h^T / 8) V_h
    out = concat_h(A_h) @ Wo.T
Sharding: tensor-parallel over heads. Core i owns heads (2i, 2i+1); the 8
partial [2048,1024] outputs are summed on the host.

v5 layout (per-core):
  * Inputs arrive via few, large HWDGE transfers (sync + scalar queues only):
    x as 4x 1MB quarter DMAs with 8KB/partition lines, weights as 256KB DMAs.
  * 16 warm-up matmuls at t=0 keep the PE busy so the HAM clock gate opens
    (2.4 GHz) before real work arrives, instead of ~24us in.
  * V is projected directly in [s, dv] orientation (lhsT = x chunk), killing
    the PE transpose pass of v4 and one round of PSUM evacuations.
  * Block-0 attention is interleaved with the per-quarter projections, so
    exp starts as soon as quarter 0 lands instead of after the full x DMA.
  * PSUM: 8 banks = scores 2x[128,1024] + AV accum 2x[128,512] +
    K/Q proj 1 + V proj 1; the proj banks are reused for the output
    projection after the projection phase ends.
  * y is written per 128-row chunk as one [128,1024] DMA into a [16,128,E]
    DRAM layout (2KB contiguous lines, host-side reshape is free).
All matmul operands bf16 (fp32 PSUM accumulation). AV rides the ones-column
trick for softmax denominators (head B offset so both normalize multiplies
stay in-lane).
"""

import numpy as np
import ml_dtypes

import concourse.mybir as mybir
import concourse.tile as tile
from concourse import bacc
from concourse.bass_utils import run_bass_kernel_spmd

S, E, H, DK, DV = 2048, 1024, 16, 64, 64
NCORES = 8
HPC = H // NCORES          # heads per core = 2
CSL = HPC * DV             # concat-dim columns per core = 128
P = 128
NE = E // P                # 8 contraction chunks for projections
SQB = 512                  # sequence block (PSUM-bank-limited matmul width)
NSQB = S // SQB            # 4
NCH = S // P               # 16 sk chunks of 128
F32 = mybir.dt.float32
BF16 = mybir.dt.bfloat16
SCALE = 1.0 / np.sqrt(DK).astype(np.float32)  # 1/8

EXP = mybir.ActivationFunctionType.Exp
MULT = mybir.AluOpType.mult

_cache = {}
last_results = None  # BassKernelResults of the most recent run (for test.py)
TRACE = False


def _build_nc():
    nc = bacc.Bacc("TRN2", target_bir_lowering=False, debug=False)

    xT = nc.dram_tensor("xT", [P, NSQB, NE, SQB], BF16, kind="ExternalInput")
    wqT = nc.dram_tensor("wqT", [P, NE, CSL], BF16, kind="ExternalInput")
    wkT = nc.dram_tensor("wkT", [P, NE, CSL], BF16, kind="ExternalInput")
    wvT = nc.dram_tensor("wvT", [P, NE, CSL], BF16, kind="ExternalInput")
    woT = nc.dram_tensor("woT", [CSL, E], BF16, kind="ExternalInput")
    y = nc.dram_tensor("y", [NCH, P, E], BF16, kind="ExternalOutput")

    xT_r = xT.ap()
    w_r = {"q": wqT.ap(), "k": wkT.ap(), "v": wvT.ap()}
    y_ap = y.ap()

    with tile.TileContext(nc) as tc:
        with tc.tile_pool(name="persist", bufs=1) as persist, \
             tc.tile_pool(name="xw", bufs=1) as xw:
            qt = persist.tile([P, S], BF16)          # QT, both heads stacked
            kpad = [
                persist.tile([P, S], BF16, name=f"kpad{h}", tag=f"kpad{h}")
                for h in range(HPC)
            ]
            # head A V-block: [V(64) | ones(2)]; head B: [32 zeros | ones(2) |
            # 30 zeros | V]: its attention output lands on partitions 64-127
            # and its denominators on 32-33 (32-aligned for custom-DVE reads)
            vaug0 = persist.tile([P, NCH, DV + 2], BF16, name="vaug0", tag="vaug0")
            vaug1 = persist.tile([P, NCH, P], BF16, name="vaug1", tag="vaug1")
            wosb = persist.tile([P, E], BF16)
            warmsb = persist.tile([P, SQB], BF16, name="warmsb", tag="warmsb")

            # ---- DMA issue first: 2 HWDGE queues, big transfers ----
            wsb = {}
            for m in ("k", "q", "v"):
                wsb[m] = xw.tile([P, NE, CSL], BF16, name=f"w{m}sb", tag=f"w{m}")
            xq = [
                xw.tile([P, NE, SQB], BF16, name=f"xq{t}", tag=f"xq{t}")
                for t in range(NSQB)
            ]
            nc.sync.dma_start(xq[0][:], xT_r[:, 0])
            nc.scalar.dma_start(wsb["k"][:], w_r["k"][:])
            nc.scalar.dma_start(wsb["q"][:], w_r["q"][:])
            nc.scalar.dma_start(wsb["v"][:], w_r["v"][:])
            nc.sync.dma_start(xq[1][:], xT_r[:, 1])
            nc.scalar.dma_start(xq[2][:], xT_r[:, 2])
            nc.sync.dma_start(xq[3][:], xT_r[:, 3])
            nc.scalar.dma_start(wosb[:], woT.ap())

            # warm the ACT exp table set right after the DMA dispatches
            warm = persist.tile([1, 16], F32, name="warm", tag="warm")
            warm2 = persist.tile([1, 16], F32, name="warm2", tag="warm2")
            nc.gpsimd.memset(warmsb[:], 0.25)
            nc.gpsimd.memset(warm[:], 0.0)
            nc.scalar.activation(warm2[:], warm[:], EXP)

            # zero/one fills (gpsimd, no DMA duties this kernel)
            nc.gpsimd.memset(kpad[0][DK:P, :], 0.0)
            nc.gpsimd.memset(kpad[1][0:DK, :], 0.0)
            nc.gpsimd.memset(vaug0[:, :, DV : DV + 2], 1.0)
            nc.gpsimd.memset(vaug1[:, :, 0:32], 0.0)
            nc.gpsimd.memset(vaug1[:, :, 32:34], 1.0)
            nc.gpsimd.memset(vaug1[:, :, 34:DV], 0.0)

            with tc.tile_pool(name="ps", bufs=1, space="PSUM") as ps, \
                 tc.tile_pool(name="est", bufs=8) as est_pool, \
                 tc.tile_pool(name="a1t", bufs=2) as a1t_pool, \
                 tc.tile_pool(name="small", bufs=6) as small, \
                 tc.tile_pool(name="outp", bufs=4) as outp:

                # ---- PE warm-up: ~16 junk matmuls from t~0 so the HAM
                # un-throttles before the first projection ----
                wps = ps.tile([P, 2 * SQB], F32, name="wps", tag="sc", bufs=2)
                for i in range(16):
                    nc.tensor.matmul(
                        wps[:, 0:SQB], lhsT=warmsb[:, 0:P], rhs=warmsb[:],
                        start=True, stop=True,
                    )

                # ---- projections (per quarter) ----
                def proj_kq(t, which):
                    sl = slice(t * SQB, (t + 1) * SQB)
                    pp = ps.tile(
                        [P, SQB], F32, name=f"p{which}{t}", tag="kq", bufs=1
                    )
                    for n in range(NE):
                        nc.tensor.matmul(
                            pp[:], lhsT=wsb[which][:, n, :], rhs=xq[t][:, n, :],
                            start=(n == 0), stop=(n == NE - 1),
                        )
                    if which == "q":
                        nc.vector.tensor_copy(qt[:, sl], pp[:])
                    else:
                        nc.vector.tensor_copy(kpad[0][0:DK, sl], pp[0:DK, :])
                        nc.vector.tensor_copy(kpad[1][DK:P, sl], pp[DK:P, :])

                def proj_v(t):
                    vp = ps.tile([P, 4, P], F32, name=f"vp{t}", tag="v", bufs=1)
                    for j in range(4):
                        for n in range(NE):
                            nc.tensor.matmul(
                                vp[:, j, :],
                                lhsT=xq[t][:, n, j * P : (j + 1) * P],
                                rhs=wsb["v"][:, n, :],
                                start=(n == 0), stop=(n == NE - 1),
                            )
                    for j in range(4):
                        c = 4 * t + j
                        nc.vector.tensor_copy(vaug0[:, c, 0:DV], vp[:, j, 0:DV])
                        nc.vector.tensor_copy(vaug1[:, c, DV:P], vp[:, j, DV:P])

                # ---- attention granules: (block b, group g of 2 chunks,
                # head h). scores -> exp (ACT) -> AV accumulate ----
                def score_granule(b, g, h):
                    bsl = slice(b * SQB, (b + 1) * SQB)
                    pss = ps.tile(
                        [P, 2 * SQB], F32, name=f"ss{b}_{g}_{h}", tag="sc",
                        bufs=2,
                    )
                    for j in range(2):
                        c = 2 * g + j
                        nc.tensor.matmul(
                            pss[:, j * SQB : (j + 1) * SQB],
                            lhsT=kpad[h][:, c * P : (c + 1) * P],
                            rhs=qt[:, bsl],
                            start=True, stop=True,
                        )
                    es = est_pool.tile(
                        [P, 2 * SQB], BF16, name=f"es{b}_{g}_{h}", tag="est"
                    )
                    nc.scalar.activation(es[:], pss[:], EXP, scale=float(SCALE))
                    return es

                def av_granule(g, h, at_ps, es):
                    for j in range(2):
                        c = 2 * g + j
                        if h == 0:
                            nc.tensor.matmul(
                                at_ps[0 : DV + 2, :],
                                lhsT=vaug0[:, c, :],
                                rhs=es[:, j * SQB : (j + 1) * SQB],
                                start=(c == 0), stop=(c == NCH - 1),
                            )
                        else:
                            nc.tensor.matmul(
                                at_ps[:],
                                lhsT=vaug1[:, c, :],
                                rhs=es[:, j * SQB : (j + 1) * SQB],
                                start=(c == 0), stop=(c == NCH - 1),
                            )

                def finish_block(b, at_ps, last):
                    # normalize: A1T rows = A^T * (1/rowsum); head A rows 0-63
                    # (denoms at 64), head B rows 64-127 (denoms at 32)
                    a1t = a1t_pool.tile([P, SQB], BF16, name=f"a1t{b}", tag="a1t")
                    rs0 = small.tile([1, SQB], F32, tag="rs0")
                    nc.vector.tensor_copy(rs0[:], at_ps[0][DV : DV + 1, :])
                    rsr0 = small.tile([1, SQB], F32, tag="rsr0")
                    nc.vector.reciprocal_approx_fast(rsr0[:], rs0[:])
                    bc0 = small.tile([P, SQB], F32, tag="bc0")
                    nc.gpsimd.partition_broadcast(bc0[:], rsr0[:])
                    nc.vector.tensor_tensor(
                        a1t[0:DV, :], at_ps[0][0:DV, :], bc0[0:DV, :], MULT
                    )
                    rs1 = small.tile([1, SQB], F32, tag="rs1")
                    nc.vector.tensor_copy(rs1[:], at_ps[1][32:33, :])
                    rsr1 = small.tile([1, SQB], F32, tag="rsr1")
                    nc.vector.reciprocal_approx_fast(rsr1[:], rs1[:])
                    bc1 = small.tile([P, SQB], F32, tag="bc1")
                    nc.gpsimd.partition_broadcast(bc1[:], rsr1[:])
                    nc.vector.tensor_tensor(
                        a1t[DV:P, :], at_ps[1][DV:P, :], bc1[DV:P, :], MULT
                    )

                    # output projection for this block: psum borrows the proj
                    # banks (kq/v tags), y DMA per 128-row chunk
                    for j in range(NSQB):
                        osb = outp.tile([P, E], BF16, tag="osb")
                        for e2 in range(E // SQB):
                            esl = slice(e2 * SQB, (e2 + 1) * SQB)
                            ops = ps.tile(
                                [P, SQB], F32, name=f"op{b}_{j}_{e2}",
                                tag=("kq" if e2 == 0 else "v"), bufs=1,
                            )
                            nc.tensor.matmul(
                                ops[:],
                                lhsT=a1t[:, j * P : (j + 1) * P],
                                rhs=wosb[:, esl],
                                start=True, stop=True,
                            )
                            if last and e2 == 0:
                                # ScalarE is idle after the last exp
                                nc.scalar.copy(osb[:, esl], ops[:])
                            else:
                                nc.vector.tensor_copy(osb[:, esl], ops[:])
                        q = nc.sync if (j % 2 == 0) else nc.scalar
                        q.dma_start(y_ap[NSQB * b + j, :, :], osb[:])

                # ---- phase 1: quarters + block-0 attention interleaved ----
                at_b0 = [
                    ps.tile([P, SQB], F32, name=f"at0_{h}", tag="av", bufs=2)
                    for h in range(HPC)
                ]
                pend = None  # (g, h, es) awaiting AV emission
                for t in range(NSQB):
                    proj_kq(t, "k")
                    if t == 0:
                        proj_kq(0, "q")
                    proj_v(t)
                    if t > 0:
                        proj_kq(t, "q")
                    for g in (2 * t, 2 * t + 1):
                        for h in range(HPC):
                            es = score_granule(0, g, h)
                            if pend is not None:
                                av_granule(pend[0], pend[1], at_b0[pend[1]], pend[2])
                            pend = (g, h, es)
                av_granule(pend[0], pend[1], at_b0[pend[1]], pend[2])
                finish_block(0, at_b0, last=False)

                # ---- phase 2: blocks 1..3 ----
                for b in range(1, NSQB):
                    at_ps = [
                        ps.tile(
                            [P, SQB], F32, name=f"at{b}_{h}", tag="av", bufs=2
                        )
                        for h in range(HPC)
                    ]
                    pend = None
                    for g in range(NCH // 2):
                        for h in range(HPC):
                            es = score_granule(b, g, h)
                            if pend is not None:
                                av_granule(pend[0], pend[1], at_ps[pend[1]], pend[2])
                            pend = (g, h, es)
                    av_granule(pend[0], pend[1], at_ps[pend[1]], pend[2])
                    finish_block(b, at_ps, last=(b == NSQB - 1))

    nc.compile()
    return nc


def kernel(x, Wq, Wk, Wv, Wo):
    global last_results
    x = np.asarray(x, dtype=np.float32)
    Wq = np.asarray(Wq, dtype=np.float32)
    Wk = np.asarray(Wk, dtype=np.float32)
    Wv = np.asarray(Wv, dtype=np.float32)
    Wo = np.asarray(Wo, dtype=np.float32)

    if "nc" not in _cache:
        _cache["nc"] = _build_nc()
    nc = _cache["nc"]

    bf = ml_dtypes.bfloat16
    # [S, E] -> [P, NSQB, NE, SQB]: xT[p, t, n, s] = x[t*SQB+s, n*P+p]
    xTq = np.ascontiguousarray(
        x.reshape(NSQB, SQB, NE, P).transpose(3, 0, 2, 1).astype(bf)
    )
    WqT = np.ascontiguousarray(Wq.T)
    WkT = np.ascontiguousarray(Wk.T)
    WvT = np.ascontiguousarray(Wv.T)
    WoT = np.ascontiguousarray(Wo.T)

    in_maps = []
    for i in range(NCORES):
        sl = slice(i * CSL, (i + 1) * CSL)

        def wslice(WT):
            # [E, CSL] slice -> [P, NE, CSL] partition-major
            return np.ascontiguousarray(
                WT[:, sl].reshape(NE, P, CSL).transpose(1, 0, 2).astype(bf)
            )

        in_maps.append({
            "xT": xTq,
            "wqT": wslice(WqT),
            "wkT": wslice(WkT),
            "wvT": wslice(WvT),
            "woT": np.ascontiguousarray(WoT[sl, :].astype(bf)),
        })

    last_results = run_bass_kernel_spmd(
        nc, in_maps, core_ids=list(range(NCORES)), trace=TRACE
    )
    out = np.zeros((S, E), dtype=np.float32)
    for r in last_results.results:
        out += r["y"].astype(np.float32).reshape(S, E)
    return out


# revision 10
# speedup vs baseline: 1.0258x; 1.0258x over previous
"""Multi-head attention TRN2 Bass kernel, head-sharded across 8 NeuronCores.

Problem: S=2048, E=1024, H=16 heads, dk=dv=64, fp32.
    Q = x @ Wq.T ; K = x @ Wk.T ; V = x @ Wv.T   (per-head slices)
    A_h = softmax(Q_h K_h^T / 8) V_h
    out = concat_h(A_h) @ Wo.T
Sharding: tensor-parallel over heads. Core i owns heads (2i, 2i+1); the 8
partial [2048,1024] outputs are summed on the host.

v6 layout (per-core):
  * Few, large HWDGE input transfers; first x quarter split across both
    queues so the first projection starts earliest. All y output DMAs ride
    the sync queue only -- a dma_start costs ~0.6us of issuing-engine time
    and the scalar engine (exp) is the kernel bottleneck.
  * 16 warm-up matmuls at t~0 open the HAM clock gate before real work.
  * Per-quarter pipeline: K_t -> scores granule -> V_t^T -> PE transposes ->
    scores granule -> Q_t, with block-0 attention interleaved so exp starts
    as soon as quarter 0 lands.
  * finish_block(b) (normalize + output projection) is emitted in the middle
    of block b+1 so its PE work fills the scores/AV pipeline instead of
    draining ACT at block boundaries.
  * PSUM: scores 2x[128,1024] + AV accum 2x[128,512] + kq 1 + v 1 = 8 banks.
  * y written per 128-row chunk as one [128,1024] DMA into a [16,128,E]
    DRAM layout (host reshape is free).
All matmul operands bf16 (fp32 PSUM accumulation). AV rides the ones-column
trick for softmax denominators (head B offset so both normalize multiplies
stay in-lane).
"""

import numpy as np
import ml_dtypes

import concourse.mybir as mybir
import concourse.tile as tile
from concourse import bacc
from concourse.bass_utils import run_bass_kernel_spmd

S, E, H, DK, DV = 2048, 1024, 16, 64, 64
NCORES = 8
HPC = H // NCORES          # heads per core = 2
CSL = HPC * DV             # concat-dim columns per core = 128
P = 128
NE = E // P                # 8 contraction chunks for projections
SQB = 512                  # sequence block (PSUM-bank-limited matmul width)
NSQB = S // SQB            # 4
NCH = S // P               # 16 sk chunks of 128
F32 = mybir.dt.float32
BF16 = mybir.dt.bfloat16
SCALE = 1.0 / np.sqrt(DK).astype(np.float32)  # 1/8

EXP = mybir.ActivationFunctionType.Exp
MULT = mybir.AluOpType.mult

_cache = {}
last_results = None  # BassKernelResults of the most recent run (for test.py)
TRACE = False


def _build_nc():
    nc = bacc.Bacc("TRN2", target_bir_lowering=False, debug=False)

    xT = nc.dram_tensor("xT", [P, NSQB, NE, SQB], BF16, kind="ExternalInput")
    wqT = nc.dram_tensor("wqT", [P, NE, CSL], BF16, kind="ExternalInput")
    wkT = nc.dram_tensor("wkT", [P, NE, CSL], BF16, kind="ExternalInput")
    wvT = nc.dram_tensor("wvT", [P, NE, CSL], BF16, kind="ExternalInput")
    woT = nc.dram_tensor("woT", [CSL, E], BF16, kind="ExternalInput")
    ident = nc.dram_tensor("ident", [P, P], BF16, kind="ExternalInput")
    y = nc.dram_tensor("y", [NCH, P, E], BF16, kind="ExternalOutput")

    xT_r = xT.ap()
    w_r = {"q": wqT.ap(), "k": wkT.ap(), "v": wvT.ap()}
    y_ap = y.ap()

    with tile.TileContext(nc) as tc:
        with tc.tile_pool(name="persist", bufs=1) as persist, \
             tc.tile_pool(name="xw", bufs=1) as xw:
            qt = persist.tile([P, S], BF16)          # QT, both heads stacked
            kpad = [
                persist.tile([P, S], BF16, name=f"kpad{h}", tag=f"kpad{h}")
                for h in range(HPC)
            ]
            vt = persist.tile([P, S], BF16, name="vt", tag="vt")
            # head A V-block: [V(64) | ones(2)]; head B: [32 zeros | ones(2) |
            # 30 zeros | V]: its attention output lands on partitions 64-127
            # and its denominators on 32-33 (32-aligned for custom-DVE reads)
            vaug0 = persist.tile([P, NCH, DV + 2], BF16, name="vaug0", tag="vaug0")
            vaug1 = persist.tile([P, NCH, P], BF16, name="vaug1", tag="vaug1")
            wosb = persist.tile([P, E], BF16)
            idsb = persist.tile([P, P], BF16, name="idsb", tag="idsb")
            warmsb = persist.tile([P, SQB], BF16, name="warmsb", tag="warmsb")

            # ---- DMA issue first: 2 HWDGE queues, big transfers.
            # First quarter is split across both queues to land earliest.
            wsb = {}
            for m in ("k", "q", "v"):
                wsb[m] = xw.tile([P, NE, CSL], BF16, name=f"w{m}sb", tag=f"w{m}")
            xq = [
                xw.tile([P, NE, SQB], BF16, name=f"xq{t}", tag=f"xq{t}")
                for t in range(NSQB)
            ]
            nc.sync.dma_start(xq[0][:, 0:4, :], xT_r[:, 0, 0:4, :])
            nc.scalar.dma_start(wsb["k"][:], w_r["k"][:])
            nc.scalar.dma_start(wsb["q"][:], w_r["q"][:])
            nc.scalar.dma_start(xq[0][:, 4:8, :], xT_r[:, 0, 4:8, :])
            nc.sync.dma_start(xq[1][:], xT_r[:, 1])
            nc.scalar.dma_start(wsb["v"][:], w_r["v"][:])
            nc.scalar.dma_start(xq[2][:], xT_r[:, 2])
            nc.sync.dma_start(idsb[:], ident.ap())
            nc.sync.dma_start(xq[3][:], xT_r[:, 3])
            nc.sync.dma_start(wosb[:], woT.ap())

            # warm the ACT exp table set right after the DMA dispatches
            warm = persist.tile([1, 16], F32, name="warm", tag="warm")
            warm2 = persist.tile([1, 16], F32, name="warm2", tag="warm2")
            nc.gpsimd.memset(warmsb[:], 0.25)
            nc.gpsimd.memset(warm[:], 0.0)
            nc.scalar.activation(warm2[:], warm[:], EXP)

            # zero/one fills (gpsimd, no DMA duties this kernel)
            nc.gpsimd.memset(kpad[0][DK:P, :], 0.0)
            nc.gpsimd.memset(kpad[1][0:DK, :], 0.0)
            nc.gpsimd.memset(vaug0[:, :, DV : DV + 2], 1.0)
            nc.gpsimd.memset(vaug1[:, :, 0:32], 0.0)
            nc.gpsimd.memset(vaug1[:, :, 32:34], 1.0)
            nc.gpsimd.memset(vaug1[:, :, 34:DV], 0.0)

            with tc.tile_pool(name="ps", bufs=1, space="PSUM") as ps, \
                 tc.tile_pool(name="est", bufs=8) as est_pool, \
                 tc.tile_pool(name="a1t", bufs=2) as a1t_pool, \
                 tc.tile_pool(name="small", bufs=6) as small, \
                 tc.tile_pool(name="outp", bufs=4) as outp:

                # ---- PE warm-up: junk matmuls from t~0 so the HAM
                # un-throttles before the first projection ----
                wps = ps.tile([P, 2 * SQB], F32, name="wps", tag="sc", bufs=2)
                for i in range(16):
                    nc.tensor.matmul(
                        wps[:, 0:SQB], lhsT=warmsb[:, 0:P], rhs=warmsb[:],
                        start=True, stop=True,
                    )

                # ---- projections (per quarter) ----
                def proj_kq(t, which):
                    sl = slice(t * SQB, (t + 1) * SQB)
                    pp = ps.tile(
                        [P, SQB], F32, name=f"p{which}{t}", tag="kq", bufs=1
                    )
                    for n in range(NE):
                        nc.tensor.matmul(
                            pp[:], lhsT=wsb[which][:, n, :], rhs=xq[t][:, n, :],
                            start=(n == 0), stop=(n == NE - 1),
                        )
                    if which == "q":
                        nc.vector.tensor_copy(qt[:, sl], pp[:])
                    else:
                        nc.vector.tensor_copy(kpad[0][0:DK, sl], pp[0:DK, :])
                        nc.vector.tensor_copy(kpad[1][DK:P, sl], pp[DK:P, :])

                def proj_vt(t):
                    sl = slice(t * SQB, (t + 1) * SQB)
                    pv = ps.tile([P, SQB], F32, name=f"pv{t}", tag="v", bufs=1)
                    for n in range(NE):
                        nc.tensor.matmul(
                            pv[:], lhsT=wsb["v"][:, n, :], rhs=xq[t][:, n, :],
                            start=(n == 0), stop=(n == NE - 1),
                        )
                    nc.vector.tensor_copy(vt[:, sl], pv[:])

                def transp_v(t):
                    vp = ps.tile(
                        [P, 4, P], BF16, name=f"vp{t}", tag="kq", bufs=1
                    )
                    for j in range(4):
                        c = 4 * t + j
                        nc.tensor.transpose(
                            vp[:, j, :], vt[:, c * P : (c + 1) * P], idsb[:]
                        )
                        nc.vector.tensor_copy(vaug0[:, c, 0:DV], vp[:, j, 0:DV])
                        nc.vector.tensor_copy(vaug1[:, c, DV:P], vp[:, j, DV:P])

                # ---- attention granules: (block b, group g of 2 chunks,
                # head h). scores -> exp (ACT) -> AV accumulate ----
                def score_granule(b, g, h):
                    bsl = slice(b * SQB, (b + 1) * SQB)
                    pss = ps.tile(
                        [P, 2 * SQB], F32, name=f"ss{b}_{g}_{h}", tag="sc",
                        bufs=2,
                    )
                    for j in range(2):
                        c = 2 * g + j
                        nc.tensor.matmul(
                            pss[:, j * SQB : (j + 1) * SQB],
                            lhsT=kpad[h][:, c * P : (c + 1) * P],
                            rhs=qt[:, bsl],
                            start=True, stop=True,
                        )
                    es = est_pool.tile(
                        [P, 2 * SQB], BF16, name=f"es{b}_{g}_{h}", tag="est"
                    )
                    nc.scalar.activation(es[:], pss[:], EXP, scale=float(SCALE))
                    return es

                def av_granule(g, h, at_ps, es):
                    for j in range(2):
                        c = 2 * g + j
                        if h == 0:
                            nc.tensor.matmul(
                                at_ps[0 : DV + 2, :],
                                lhsT=vaug0[:, c, :],
                                rhs=es[:, j * SQB : (j + 1) * SQB],
                                start=(c == 0), stop=(c == NCH - 1),
                            )
                        else:
                            nc.tensor.matmul(
                                at_ps[:],
                                lhsT=vaug1[:, c, :],
                                rhs=es[:, j * SQB : (j + 1) * SQB],
                                start=(c == 0), stop=(c == NCH - 1),
                            )

                def finish_block(b, at_ps, last):
                    # normalize: A1T rows = A^T * (1/rowsum); head A rows 0-63
                    # (denoms at 64), head B rows 64-127 (denoms at 32);
                    # reciprocals straight from PSUM
                    a1t = a1t_pool.tile([P, SQB], BF16, name=f"a1t{b}", tag="a1t")
                    rs0 = small.tile([1, SQB], F32, tag="rs0")
                    nc.vector.tensor_copy(rs0[:], at_ps[0][DV : DV + 1, :])
                    rsr0 = small.tile([1, SQB], F32, tag="rsr0")
                    nc.vector.reciprocal_approx_fast(rsr0[:], rs0[:])
                    bc0 = small.tile([P, SQB], F32, tag="bc0")
                    nc.gpsimd.partition_broadcast(bc0[:], rsr0[:])
                    rs1 = small.tile([1, SQB], F32, tag="rs1")
                    nc.vector.tensor_copy(rs1[:], at_ps[1][32:33, :])
                    rsr1 = small.tile([1, SQB], F32, tag="rsr1")
                    nc.vector.reciprocal_approx_fast(rsr1[:], rs1[:])
                    bc1 = small.tile([P, SQB], F32, tag="bc1")
                    nc.gpsimd.partition_broadcast(bc1[:], rsr1[:])
                    nc.vector.tensor_tensor(
                        a1t[0:DV, :], at_ps[0][0:DV, :], bc0[0:DV, :], MULT
                    )
                    nc.vector.tensor_tensor(
                        a1t[DV:P, :], at_ps[1][DV:P, :], bc1[DV:P, :], MULT
                    )

                    # output projection for this block: psum borrows the proj
                    # banks (kq/v tags; + sc for the last block), y DMA per
                    # 128-row chunk on the sync queue
                    for j in range(NSQB):
                        osb = outp.tile([P, E], BF16, tag="osb")
                        for e2 in range(E // SQB):
                            esl = slice(e2 * SQB, (e2 + 1) * SQB)
                            if last and e2 == 0:
                                ops = ps.tile(
                                    [P, SQB], F32, name=f"op{b}_{j}_{e2}",
                                    tag="sc", bufs=2,
                                )
                            else:
                                ops = ps.tile(
                                    [P, SQB], F32, name=f"op{b}_{j}_{e2}",
                                    tag=("kq" if e2 == 0 else "v"), bufs=1,
                                )
                            nc.tensor.matmul(
                                ops[:],
                                lhsT=a1t[:, j * P : (j + 1) * P],
                                rhs=wosb[:, esl],
                                start=True, stop=True,
                            )
                            if last and e2 == 0:
                                # ScalarE is idle after the last exp
                                nc.scalar.copy(osb[:, esl], ops[:])
                            else:
                                nc.vector.tensor_copy(osb[:, esl], ops[:])
                        nc.sync.dma_start(y_ap[NSQB * b + j, :, :], osb[:])

                # ---- phase 1: quarters + block-0 attention interleaved ----
                from collections import deque

                at_tiles = {}
                at_tiles[0] = [
                    ps.tile([P, SQB], F32, name=f"at0_{h}", tag="av", bufs=2)
                    for h in range(HPC)
                ]
                pend = deque()  # (b, g, h, es) awaiting AV emission

                def emit_scores(b, g, h):
                    pend.append((b, g, h, score_granule(b, g, h)))

                def flush_one():
                    pb, pg, ph, pes = pend.popleft()
                    av_granule(pg, ph, at_tiles[pb][ph], pes)

                def emit_flush(b, g, h):
                    # steady state: emit scores granule i+1, then AV of i
                    emit_scores(b, g, h)
                    while len(pend) > 1:
                        flush_one()

                for t in range(NSQB):
                    proj_kq(t, "k")
                    if t == 0:
                        proj_kq(0, "q")
                    emit_flush(0, 2 * t, 0)
                    # no flush: av of (2t, 0) must wait for this quarter's
                    # V transposes (it reads vaug chunks 4t, 4t+1)
                    emit_scores(0, 2 * t, 1)
                    proj_vt(t)
                    transp_v(t)
                    emit_flush(0, 2 * t + 1, 0)
                    emit_flush(0, 2 * t + 1, 1)
                    if t > 0:
                        proj_kq(t, "q")
                    # pend leaves each quarter with exactly one entry

                # ---- phase 2: blocks 1..3. finish of block b-1 is emitted
                # after three score granules of block b (all its at-psum
                # reads must precede block b's first AV write, which reuses
                # the same psum slots), so its outproj fills the PE while
                # ACT churns through the queued exps ----
                for b in range(1, NSQB):
                    emit_scores(b, 0, 0)
                    flush_one()  # av of (b-1, 7, 1): completes block b-1
                    emit_scores(b, 0, 1)
                    emit_scores(b, 1, 0)
                    finish_block(b - 1, at_tiles[b - 1], last=False)
                    # allocate AFTER finish_block so the psum-slot reuse
                    # sees the normalize reads of block b-1
                    at_tiles[b] = [
                        ps.tile(
                            [P, SQB], F32, name=f"at{b}_{h}", tag="av", bufs=2
                        )
                        for h in range(HPC)
                    ]
                    flush_one()  # av of (b, 0, 0)
                    flush_one()  # av of (b, 0, 1)
                    emit_flush(b, 1, 1)
                    for g in range(2, NCH // 2):
                        for h in range(HPC):
                            emit_flush(b, g, h)
                flush_one()
                finish_block(NSQB - 1, at_tiles[NSQB - 1], last=True)

    nc.compile()
    return nc


def kernel(x, Wq, Wk, Wv, Wo):
    global last_results
    x = np.asarray(x, dtype=np.float32)
    Wq = np.asarray(Wq, dtype=np.float32)
    Wk = np.asarray(Wk, dtype=np.float32)
    Wv = np.asarray(Wv, dtype=np.float32)
    Wo = np.asarray(Wo, dtype=np.float32)

    if "nc" not in _cache:
        _cache["nc"] = _build_nc()
    nc = _cache["nc"]

    bf = ml_dtypes.bfloat16
    # [S, E] -> [P, NSQB, NE, SQB]: xT[p, t, n, s] = x[t*SQB+s, n*P+p]
    xTq = np.ascontiguousarray(
        x.reshape(NSQB, SQB, NE, P).transpose(3, 0, 2, 1).astype(bf)
    )
    WqT = np.ascontiguousarray(Wq.T)
    WkT = np.ascontiguousarray(Wk.T)
    WvT = np.ascontiguousarray(Wv.T)
    WoT = np.ascontiguousarray(Wo.T)

    in_maps = []
    for i in range(NCORES):
        sl = slice(i * CSL, (i + 1) * CSL)

        def wslice(WT):
            # [E, CSL] slice -> [P, NE, CSL] partition-major
            return np.ascontiguousarray(
                WT[:, sl].reshape(NE, P, CSL).transpose(1, 0, 2).astype(bf)
            )

        in_maps.append({
            "xT": xTq,
            "ident": np.eye(P, dtype=np.float32).astype(bf),
            "wqT": wslice(WqT),
            "wkT": wslice(WkT),
            "wvT": wslice(WvT),
            "woT": np.ascontiguousarray(WoT[sl, :].astype(bf)),
        })

    last_results = run_bass_kernel_spmd(
        nc, in_maps, core_ids=list(range(NCORES)), trace=TRACE
    )
    out = np.zeros((S, E), dtype=np.float32)
    for r in last_results.results:
        out += r["y"].astype(np.float32).reshape(S, E)
    return out
